# revision 53
# baseline (speedup 1.0000x reference)
"""PerceptualMelLoss on 8 trn2 NeuronCores.

Data-parallel over batch (8 items/core). Each core computes masked partial
sums for all six loss reductions; host finishes the weighted formula in f64.

Device layout per core: 8 items -> 2 groups of 4 items. A group's
4*4000 = 16000 frames map to [128 partitions x 125 frames]; item boundaries
fall exactly on partition boundaries (4000 = 32*125). Compute tiles are
d-major bf16 [128, 80 mels, 125 frames] so the per-frame mask broadcasts
with a stride-0 middle dim (keeps DVE 2x/4x modes, which require the
innermost dim contiguous).

The whole pipeline is chunked in 5-21 frame j-slices of the group tiles so
DMA, Pool, DVE and ACT stream concurrently; chunk seams need no special
handling because neighbours' columns live in the same tile. Only the
cross-partition frame shift (previous frame of partition p's first column)
is deferred to group end, done with a PE matmul against a host-supplied
shift matrix whose item-boundary columns are zero.

Engine assignment (per-core busy, vs ~58 us DMA floor). Only
compiler-encodable reduction forms are used: the ACT accumulator
(activation func + accum_out) and DVE tensor_reduce with
apply_absolute_value; tensor_scalar/TTR accumulate variants are rejected
by this neuronxcc version.
  Pool   E = P - T f32 chunks + part of T^2 (tensor_tensor) (~56 us)
  DVE    Em = E*mask, D1/D2 shift-subs (2x), most of |D1| and |R| via
         tensor_reduce(abs), energy tree-reduce (~64 us)
  ACT    most T^2 chunks, fused-accumulator reductions: |Em| by band,
         |D2|, part of |D1| (~64 us)
  PE     masked sum m*T^2 via mask-column matmuls; sum m*E^2 as the
         diagonal of an accumulated Em-Gram (m^2 = m, host sums the
         trace); cross-partition frame shift (~29 us)

Reductions land in per-partition accum columns of an f32 [128, 128]
tile, DMA'd out raw; the host does the 128-way sums in f64.

Approximations (all << 2e-2 tolerance): bf16 elementwise math; delta terms
use tail-zeroed Em, so each item's mask edge contributes ~1 spurious delta
(~2e-4 relative on the delta sums).
"""

import numpy as np

import bass_rust as _bass_rust
import concourse.bass as bass
import concourse.tile as tile
from concourse.bass_utils import run_bass_kernel_spmd
from concourse import mybir

NCORES = 8
B, T, D = 64, 4000, 80
BPC = B // NCORES          # items per core
G = 2                      # groups per core
IPG = 4                    # items per group
P, J = 128, 125            # group frames = P*J = 16000
CHS = (5, 20, 21, 21, 21, 21, 16)  # small edge chunks: fast fill, short tail
# chunks are PROCESSED high-j -> low-j so the cross-partition shift (frame
# col 124) leaves the critical tail; D1/D2 lag the stream by 1-2 columns
NCH = len(CHS)
CHB = tuple(sum(CHS[:i]) for i in range(NCH + 1))   # chunk j-offsets
CHMX = max(CHS)
SQT_POOL = (0, 1, 3)       # Square(T) chunks computed on Pool (rest on ACT)
SUPER = ((6, 5), (4, 3), (2, 1), (0,))   # chunk pairs for reduction ops

F32 = mybir.dt.float32
BF16 = mybir.dt.bfloat16
ALU = mybir.AluOpType
AF = mybir.ActivationFunctionType
AX = mybir.AxisListType

W_L1, W_DELTA, W_DELTA2, W_SC, W_BAND, W_ENERGY = 1.0, 0.5, 0.25, 0.5, 1.0, 0.5
F_LO, F_HI = 10, 50        # formant band
EPS = 1e-8

_NC = None


def _build_nc():
    nc = bass.Bass()
    pred = nc.dram_tensor("pred", [BPC, T, D], F32, kind="ExternalInput")
    targ = nc.dram_tensor("targ", [BPC, T, D], F32, kind="ExternalInput")
    mask = nc.dram_tensor("mask", [BPC, T], F32, kind="ExternalInput")
    # cols 0..P-1: shift matrix, smat[p, m] = 1 iff m = p+1 and m % 32 != 0
    # (item starts excluded), so (smat^T @ x)[m] = x[m-1] with zeros at item
    # boundaries. col P: 0.0 at item-start partitions, 1.0 elsewhere.
    smat = nc.dram_tensor("smat", [P, P + 1], BF16, kind="ExternalInput")
    acc_out = nc.dram_tensor("acc", [P, 128], F32, kind="ExternalOutput")
    sden_out = nc.dram_tensor("sden", [1, D], F32, kind="ExternalOutput")
    snum_out = nc.dram_tensor("snum", [D, D], F32, kind="ExternalOutput")

    with tile.TileContext(nc) as tc, \
         tc.tile_pool(name="persist", bufs=1) as ppool, \
         tc.tile_pool(name="pchunk", bufs=3) as pcpool, \
         tc.tile_pool(name="tchunk", bufs=3) as tcpool, \
         tc.tile_pool(name="escr", bufs=3) as epool, \
         tc.tile_pool(name="qt", bufs=2) as qtpool, \
         tc.tile_pool(name="em", bufs=2) as empool, \
         tc.tile_pool(name="d1", bufs=1) as d1pool, \
         tc.tile_pool(name="d2", bufs=1) as d2pool, \
         tc.tile_pool(name="ascr", bufs=1) as apool, \
         tc.tile_pool(name="tree", bufs=1) as trpool, \
         tc.tile_pool(name="small", bufs=2) as smpool, \
         tc.tile_pool(name="psum", bufs=2,
                      space=bass.MemorySpace.PSUM) as pspool, \
         tc.tile_pool(name="psum1", bufs=1,
                      space=bass.MemorySpace.PSUM) as ps1pool:

        S_t = ppool.tile([P, P + 1], BF16, name="S_t")
        mt = ppool.tile([P, G, J], BF16, name="mt")
        acc = ppool.tile([P, 128], F32, name="acc")
        stage = ppool.tile([1, D], F32, name="stage")
        psden = ps1pool.tile([1, D], F32, name="psden")
        psnum = ps1pool.tile([D, D], F32, name="psnum")
        stage2 = ppool.tile([D, D], F32, name="stage2")

        nc.vector.memset(acc[:], 0.0)

        for g in range(G):
            pg = pred[IPG * g:IPG * (g + 1)].rearrange(
                "b (pb j) d -> (b pb) j d", pb=P // IPG)
            tg = targ[IPG * g:IPG * (g + 1)].rearrange(
                "b (pb j) d -> (b pb) j d", pb=P // IPG)
            mg = mask[IPG * g:IPG * (g + 1)].rearrange(
                "b (pb j) -> (b pb) j", pb=P // IPG)

            QT = qtpool.tile([P, J, D], BF16, name="QT")         # frame-major
            Em = empool.tile([P, D, J], BF16, name="Em")
            D1m = d1pool.tile([P, D, J], BF16, name="D1m")
            D2m = d2pool.tile([P, D, J], BF16, name="D2m")
            Ascr = apool.tile([P, D, 42], BF16, name="Ascr")     # ACT discard
            Tr = trpool.tile([P, 40, J], BF16, name="Tr")
            Tr2 = trpool.tile([P, 25, J], BF16, name="Tr2")
            ac = lambda k: acc[:, 64 * g + k:64 * g + k + 1]
            mf = smpool.tile([P, J], F32, name="mf")
            zc = S_t[:, P:P + 1]     # 0.0 at item-start partitions
            EmP = smpool.tile([P, D], BF16, name="EmP")
            D1P = smpool.tile([P, D], BF16, name="D1P")

            first = True
            deferred_sqt = []
            for si, sc in enumerate(SUPER):
                for c in sc:
                    j0 = CHB[c]
                    CH = CHS[c]
                    js = slice(j0, j0 + CH)
                    Pc = pcpool.tile([P, CHMX, D], F32, name="Pc")
                    Tc = tcpool.tile([P, CHMX, D], F32, name="Tc")
                    nc.sync.dma_start(out=Pc[:, 0:CH, :], in_=pg[:, js, :])
                    nc.sync.dma_start(out=Tc[:, 0:CH, :], in_=tg[:, js, :])
                    if first:
                        # small DMAs after the first data chunks are in flight
                        nc.sync.dma_start(out=mf[:], in_=mg)
                        nc.vector.tensor_scalar(mt[:, g, :], mf[:], 0.0, None,
                                                op0=ALU.add)
                        if g == 0:
                            nc.sync.dma_start(out=S_t[:], in_=smat[:, :])
                    # E chunk, written transposed into a d-major scratch
                    Escr = epool.tile([P, D, CHMX], BF16, name="Escr")
                    nc.gpsimd.tensor_tensor(
                        Escr[:, :, 0:CH].transpose([0, 2, 1]), Pc[:, 0:CH, :],
                        Tc[:, 0:CH, :], op=ALU.subtract)
                    if c in SQT_POOL:
                        if si >= len(SUPER) - 2:
                            # defer: keep Pool free for the tail's E chunks
                            deferred_sqt.append((js, CH, Pc, Tc))
                        else:
                            nc.gpsimd.tensor_tensor(QT[:, js, :],
                                                    Tc[:, 0:CH, :],
                                                    Tc[:, 0:CH, :],
                                                    op=ALU.mult)
                    else:
                        nc.scalar.activation(QT[:, js, :], Tc[:, 0:CH, :],
                                             AF.Square)

                    mbc = mt[:, g, js].unsqueeze(1).broadcast_to((P, D, CH))
                    nc.vector.tensor_tensor(Em[:, :, js], Escr[:, :, 0:CH],
                                            mbc, op=ALU.mult)

                    # sum m*E^2 on the PE: diag of the accumulated Em Gram
                    # (m^2 = m, so Em.Em = m*E^2); host sums the diagonal
                    for j in range(j0, j0 + CH):
                        nc.tensor.matmul(psnum[:], Em[:, :, j], Em[:, :, j],
                                         start=(g == 0 and c == NCH - 1
                                                and j == j0),
                                         stop=(g == G - 1 and c == 0
                                               and j == j0 + CH - 1))

                    if first:
                        # frame col J-1 is available: cross-partition shift
                        psA = pspool.tile([P, D], F32, name="psA")
                        nc.tensor.matmul(psA[:], S_t[:, 0:P], Em[:, :, J - 1],
                                         start=True, stop=True)
                        nc.vector.tensor_scalar(EmP[:], psA[:], 0.0, None,
                                                op0=ALU.add)
                        first = False

                    # masked sum of T^2 on the PE (mask column stationary)
                    if not (c in SQT_POOL and si >= len(SUPER) - 2):
                        for j in range(j0, j0 + CH):
                            nc.tensor.matmul(psden[:], mt[:, g, j:j + 1],
                                             QT[:, j, :],
                                             start=(g == 0 and c == NCH - 1
                                                    and j == j0),
                                             stop=False)

                # ---- super-chunk (pair) level ops ----
                jlo = CHB[sc[-1]]
                jhi = CHB[sc[0] + 1]
                ps = slice(jlo, jhi)
                W = jhi - jlo

                # deltas lag the (reversed) stream by 1-2 columns
                d1s = slice(jlo + 1, min(jhi + 1, J))
                nc.vector.tensor_tensor(
                    D1m[:, :, d1s], Em[:, :, d1s],
                    Em[:, :, d1s.start - 1:d1s.stop - 1], op=ALU.subtract)
                if si == 0:
                    psB = pspool.tile([P, D], F32, name="psB")
                    nc.tensor.matmul(psB[:], S_t[:, 0:P], D1m[:, :, J - 1],
                                     start=True, stop=True)
                    nc.vector.tensor_scalar(D1P[:], psB[:], 0.0, None,
                                            op0=ALU.add)
                d2s = slice(jlo + 2, min(jhi + 2, J))
                nc.vector.tensor_tensor(
                    D2m[:, :, d2s], D1m[:, :, d2s],
                    D1m[:, :, d2s.start - 1:d2s.stop - 1], op=ALU.subtract)

                # fused DVE reductions (4x): |Em| by band, sum Em^2;
                # deferred for the last two pairs to keep the delta chain hot
                def emit_dve_reductions(si=si, ps=ps, W=W):
                    nc.vector.tensor_scalar(Vscr[:, 0:F_LO, 0:W],
                                            Em[:, 0:F_LO, ps], 0.0, 0.0,
                                            op0=ALU.abs_max, op1=ALU.add,
                                            accum_out=ac(9 * si))
                    nc.vector.tensor_scalar(Vscr[:, F_LO:F_HI, 0:W],
                                            Em[:, F_LO:F_HI, ps], 0.0, 0.0,
                                            op0=ALU.abs_max, op1=ALU.add,
                                            accum_out=ac(9 * si + 1))
                    nc.vector.tensor_scalar(Vscr[:, F_HI:D, 0:W],
                                            Em[:, F_HI:D, ps], 0.0, 0.0,
                                            op0=ALU.abs_max, op1=ALU.add,
                                            accum_out=ac(9 * si + 2))
                    nc.scalar.activation(Ascr[:, :, 0:W], Em[:, :, ps],
                                         AF.Square, accum_out=ac(9 * si + 3))
                emit_dve_reductions()

                nc.vector.tensor_scalar(Vscr[:, :, 0:d1s.stop - d1s.start],
                                        D1m[:, :, d1s], 0.0, 0.0,
                                        op0=ALU.abs_max, op1=ALU.add,
                                        accum_out=ac(9 * si + 4))
                # ACT reduction with hardware accumulator
                nc.scalar.activation(Ascr[:, :, 0:d2s.stop - d2s.start],
                                     D2m[:, :, d2s], AF.Abs,
                                     accum_out=ac(9 * si + 5))

                # energy tree, levels 1-2 (rest at group end)
                nc.vector.tensor_tensor(Tr[:, 0:40, ps], Em[:, 0:40, ps],
                                        Em[:, 40:80, ps], op=ALU.add)
                nc.vector.tensor_tensor(Tr2[:, 0:20, ps], Tr[:, 0:20, ps],
                                        Tr[:, 20:40, ps], op=ALU.add)

            # ---- group end: leading boundary columns (tiny) ----
            # (Em[:, :, 0] * zc) - EmP: zeroes the item-start columns since
            # the shift matrix's item-start columns are zero too
            nc.vector.scalar_tensor_tensor(D1m[:, :, 0:1], Em[:, :, 0:1],
                                           zc, EmP[:].unsqueeze(2),
                                           op0=ALU.mult, op1=ALU.subtract)
            nc.vector.tensor_tensor(D2m[:, :, 0:1], D1m[:, :, 0:1],
                                    D1P[:].unsqueeze(2), op=ALU.subtract)
            # frame 1 of each item has no valid 2nd delta
            nc.vector.scalar_tensor_tensor(D2m[:, :, 1:2], D1m[:, :, 1:2],
                                           zc, D1m[:, :, 0:1],
                                           op0=ALU.mult, op1=ALU.subtract)
            nc.vector.tensor_reduce(ac(61), D1m[:, :, 0:1], axis=AX.XY,
                                    op=ALU.add, apply_absolute_value=True)
            nc.vector.tensor_reduce(ac(62), D2m[:, :, 0:2], axis=AX.XY,
                                    op=ALU.add, apply_absolute_value=True)

            for js_, CH_, Pc_, Tc_ in deferred_sqt:
                nc.gpsimd.tensor_tensor(QT[:, js_, :], Tc_[:, 0:CH_, :],
                                        Tc_[:, 0:CH_, :], op=ALU.mult)
                for j in range(js_.start, js_.stop):
                    nc.tensor.matmul(psden[:], mt[:, g, j:j + 1], QT[:, j, :],
                                     start=False,
                                     stop=(g == G - 1 and j == js_.stop - 1
                                           and js_.start == 0))
            # energy tree levels 3+ over the whole group, then |R| accum
            nc.vector.tensor_tensor(Tr[:, 0:10, :], Tr2[:, 0:10, :],
                                    Tr2[:, 10:20, :], op=ALU.add)
            nc.vector.tensor_tensor(Tr2[:, 20:25, :], Tr[:, 0:5, :],
                                    Tr[:, 5:10, :], op=ALU.add)
            nc.vector.tensor_tensor(Tr[:, 10:12, :], Tr2[:, 20:22, :],
                                    Tr2[:, 22:24, :], op=ALU.add)
            nc.vector.tensor_tensor(Tr[:, 12:13, :], Tr[:, 10:11, :],
                                    Tr[:, 11:12, :], op=ALU.add)
            nc.vector.tensor_tensor(Tr[:, 13:14, :], Tr[:, 12:13, :],
                                    Tr2[:, 24:25, :], op=ALU.add)
            nc.vector.tensor_scalar(Tr[:, 14, :], Tr[:, 13, :],
                                    0.0, 0.0, op0=ALU.abs_max, op1=ALU.add,
                                    accum_out=ac(63))


        nc.vector.tensor_scalar(stage[:], psden[:], 0.0, None, op0=ALU.add)
        nc.sync.dma_start(out=sden_out[:], in_=stage[:])
        nc.vector.tensor_scalar(stage2[:], psnum[:], 0.0, None, op0=ALU.add)
        nc.sync.dma_start(out=snum_out[:], in_=stage2[:])
        nc.sync.dma_start(out=acc_out[:], in_=acc[:])

    # TRN2 allows at most one semaphore wait per instruction; this pass
    # splits multi-wait instructions via InstEventSemaphore.
    _bass_rust.generate_event_semaphores(nc)
    return nc


def _make_smat():
    bf16 = mybir.dt.np(BF16)
    S = np.zeros((P, P + 1), dtype=bf16)
    for m in range(1, P):
        if m % (P // IPG) != 0:
            S[m - 1, m] = 1.0
    for p in range(P):
        S[p, P] = 0.0 if p % (P // IPG) == 0 else 1.0
    return S


def make_in_maps(pred_mel, target_mel, mel_mask):
    pred_mel = np.ascontiguousarray(pred_mel, dtype=np.float32)
    target_mel = np.ascontiguousarray(target_mel, dtype=np.float32)
    mel_mask = np.ascontiguousarray(mel_mask, dtype=np.float32)
    smat = _make_smat()
    in_maps = []
    for c in range(NCORES):
        s = slice(c * BPC, (c + 1) * BPC)
        in_maps.append({
            "pred": pred_mel[s],
            "targ": target_mel[s],
            "mask": mel_mask[s],
            "smat": smat,
        })
    return in_maps


def kernel(pred_mel, target_mel, mel_mask, band_weights):
    global _NC
    if _NC is None:
        _NC = _build_nc()

    in_maps = make_in_maps(pred_mel, target_mel, mel_mask)
    res = run_bass_kernel_spmd(_NC, in_maps, list(range(NCORES)))

    acc = np.zeros((P, 128), dtype=np.float64)
    s_den = 0.0
    s_num = 0.0
    for r in res.results:
        acc += r["acc"].astype(np.float64)
        s_den += float(r["sden"].astype(np.float64).sum())
        s_num += float(np.trace(r["snum"].astype(np.float64)))
    cols = acc.sum(axis=0)          # [128]

    def q(k):                       # sum a quantity over groups and super-chunks
        return sum(cols[64 * g + 9 * s + k] for g in range(G)
                   for s in range(len(SUPER)))

    def qg(k):                      # group-level column
        return cols[k] + cols[64 + k]

    s1_lo, s1_for, s1_hi = q(0), q(1), q(2)
    s1 = s1_lo + s1_for + s1_hi
    s_d = q(3) + q(4) + qg(61)
    s_d2 = q(5) + qg(62)
    s_en = qg(63)

    m = np.ascontiguousarray(mel_mask, dtype=np.float32).astype(np.float64)
    cm = m.sum()
    cd = (m[:, 1:] * m[:, :-1]).sum()
    cd2 = (m[:, 2:] * m[:, 1:-1] * m[:, :-2]).sum()

    n1 = max(D * cm, 1.0)
    l1 = s1 / n1
    delta = s_d / max(D * cd, 1.0)
    delta2 = s_d2 / max(D * cd2, 1.0)
    sc = np.sqrt(s_num / n1) / max(np.sqrt(s_den / n1), EPS)
    w = band_weights.astype(np.float64)
    w_out, w_in = float(w[0]), float(w[F_LO])
    band = (w_out * (s1_lo + s1_hi) + w_in * s1_for) / n1 / w.mean()
    energy = (s_en / D) / max(cm, 1.0)

    total = (W_L1 * l1 + W_DELTA * delta + W_DELTA2 * delta2
             + W_SC * sc + W_BAND * band + W_ENERGY * energy)
    return np.float32(total)
